# revision 1
# baseline (speedup 1.0000x reference)
"""Trainium2 Bass kernel for Conv2Demod (StyleGAN2-style modulated conv).

Reference computation (per sample b):
    w[b,o,i,ky,kx] = weight[o,i,ky,kx] * (1 + s[b,i])
    d[b,o]         = rsqrt(sum_{i,ky,kx} w^2 + 1e-8)
    out[b]         = conv2d(img[b], w[b]*d[b,o], pad=1)

Key algebraic restructuring used here:
  - The (1+s) modulation is folded into the weights on-chip (per-partition
    scale in the [i, o] transposed layout).
  - The demodulation d[b,o] is applied as a per-partition scale while
    evicting conv results from PSUM (free).
  - sum w^2 = (sum_k weight^2)^T @ (1+s)^2 -- a tiny on-device matvec using
    the host-precomputed static tensor A_T[i,o] = sum_k weight[o,i,:,:]^2.
  - The weight is pre-transposed on the host to [ky*kx, i, o] so the matmul
    lhsT tiles ([i, o] with i on partitions) DMA contiguously; the conv
    contraction runs over i on the PE.

Sharding: data-parallel over batch -- 8 samples onto 8 NeuronCores, one
sample per core; weight inputs replicated.

Conv as matmul: out[o, y0:y0+8, :] accumulates 36 PE matmuls (4 i-tiles x 9
kernel positions) of lhsT=[128i, 128o] x rhs=[128i, 8x64 pixels] into one
PSUM bank, reading the rhs from a zero-padded [128, 66, 66] image tile with
a strided access pattern. float32r operands give full PE rate (1 row/cycle)
at ~1.5e-4 relative error.
"""

import contextlib

import numpy as np

import concourse.bacc as bacc
import concourse.mybir as mybir
import concourse.tile as tile
from concourse.bass_utils import run_bass_kernel_spmd

P = 128          # partitions
CIN = 512
COUT = 512
H = W = 64
KS = 3
KYX = KS * KS    # 9 kernel positions
NI = CIN // P    # 4 i-tiles
NO = COUT // P   # 4 o-tiles
RCH = 8          # output rows per chunk
NCH = H // RCH   # 8 chunks
NPIX = RCH * W   # 512 = matmul N
HP = H + 2       # padded
WP = W + 2
EPS = 1e-8
N_CORES = 8

F32 = mybir.dt.float32
F32R = mybir.dt.float32r
AF = mybir.ActivationFunctionType
_nullcm = contextlib.nullcontext


def build_nc(chain=False, loop_n=None):
    """Per-core program: one sample's modulated conv.

    chain=True adds a tiny s->s_out DRAM copy output used by timing.py to
    build a data-dependent on-device repetition chain. loop_n wraps the
    whole body in a hardware For loop (timing only).
    """
    nc = bacc.Bacc("TRN2", target_bir_lowering=False, debug=False)

    # host-padded image: [i_tile, partition, 66, 66] with zero border
    img = nc.dram_tensor("img", [NI, P, HP, WP], F32, kind="ExternalInput").ap()
    s_in = nc.dram_tensor("s", [CIN], F32, kind="ExternalInput").ap()
    wt = nc.dram_tensor("wt", [KYX, CIN, COUT], F32, kind="ExternalInput").ap()
    at = nc.dram_tensor("at", [CIN, COUT], F32, kind="ExternalInput").ap()
    out = nc.dram_tensor("out", [COUT, H, W], F32, kind="ExternalOutput").ap()
    s_out = None
    if chain:
        s_out = nc.dram_tensor("s_out", [CIN], F32, kind="ExternalOutput").ap()

    with tile.TileContext(nc) as tc:
        with (
            tc.tile_pool(name="const", bufs=1) as cpool,
            tc.tile_pool(name="persist", bufs=1) as ppool,
            tc.tile_pool(name="wstage", bufs=4) as wstage,
            tc.tile_pool(name="outsb", bufs=4) as outsb,
            tc.tile_pool(name="psum", bufs=6, space="PSUM") as psum_pool,
            tc.tile_pool(name="psum_d", bufs=1, space="PSUM") as psum_d,
        ):
            with (tc.For_i(0, loop_n, 1) if loop_n else _nullcm()):
                # ---- s-derived scalars -------------------------------------
                # duplicated column pairs so the f32r matvec rhs has an even
                # innermost count (fp32r ISA restriction)
                sraw = cpool.tile([P, NI, 2], F32, tag="sraw")
                for c in range(2):
                    nc.sync.dma_start(
                        sraw[:, :, c], s_in.rearrange("(t p) -> p t", p=P)
                    )
                if chain:
                    nc.sync.dma_start(s_out[:], s_in[:])
                smod = cpool.tile([P, NI, 2], F32, tag="smod")  # 1 + s
                nc.scalar.activation(smod[:], sraw[:], AF.Copy, bias=1.0)
                tsq = cpool.tile([P, NI, 2], F32R, tag="tsq")   # (1 + s)^2
                nc.scalar.square(tsq[:], smod[:])

                # ---- demod d[o] = 1/sqrt(A_T.T @ tsq + eps) ----------------
                at_sb = ppool.tile([P, NI, COUT], F32R, tag="at_sb")
                nc.sync.dma_start(
                    at_sb[:], at.rearrange("(t p) o -> p t o", p=P).bitcast(F32R)
                )
                dsb = cpool.tile([P, NO], F32, tag="dsb")
                dtmp = cpool.tile([P, NO], F32, tag="dtmp")
                epst = cpool.tile([P, 1], F32, tag="epst")
                nc.vector.memset(epst[:], EPS)
                for ot in range(NO):
                    o0 = ot * P
                    psd = psum_d.tile([P, 2], F32)
                    for it in range(NI):
                        nc.tensor.matmul(
                            psd[:],
                            at_sb[:, it, o0 : o0 + P],
                            tsq[:, it, :],
                            start=(it == 0),
                            stop=(it == NI - 1),
                        )
                    nc.scalar.activation(
                        dtmp[:, ot : ot + 1], psd[:, 0:1], AF.Sqrt, bias=epst[:]
                    )
                nc.vector.reciprocal(dsb[:], dtmp[:])

                # ---- padded image tiles (padding done host-side) -----------
                imgsb = []
                for it in range(NI):
                    t = ppool.tile([P, HP, WP], F32R, tag=f"imgsb{it}")
                    imgsb.append(t)
                    nc.sync.dma_start(t[:], img[it].bitcast(F32R))

                # ---- modulated transposed weights --------------------------
                wmod = []
                for it in range(NI):
                    t = ppool.tile([P, KYX, COUT], F32R, tag=f"wmod{it}")
                    wmod.append(t)
                    for kyx in range(KYX):
                        wst = wstage.tile([P, COUT], F32)
                        nc.sync.dma_start(
                            wst[:], wt[kyx, it * P : (it + 1) * P, :]
                        )
                        nc.scalar.activation(
                            t[:, kyx, :], wst[:], AF.Copy,
                            scale=smod[:, it, 0:1],
                        )

                # ---- conv: 4 o-tiles x 8 chunks x 36 accumulating matmuls --
                # Out-of-bounds kernel taps are clipped: PSUM's per-element
                # has_written bit makes the first writer of each element
                # overwrite and later writers accumulate, so skipped border
                # contributions behave as zero padding.
                for ot in range(NO):
                    o0 = ot * P
                    for ch in range(NCH):
                        y0 = ch * RCH
                        ps = psum_pool.tile([P, NPIX], F32)
                        n_mm = NI * KYX
                        j = 0
                        for it in range(NI):
                            for kyx in range(KYX):
                                ky, kx = kyx // KS, kyx % KS
                                nc.tensor.matmul(
                                    ps[:],
                                    wmod[it][:, kyx, o0 : o0 + P],
                                    imgsb[it][
                                        :, y0 + ky : y0 + ky + RCH, kx : kx + W
                                    ],
                                    start=(j == 0),
                                    stop=(j == n_mm - 1),
                                )
                                j += 1
                        osb = outsb.tile([P, NPIX], F32)
                        nc.scalar.activation(
                            osb[:], ps[:], AF.Copy, scale=dsb[:, ot : ot + 1]
                        )
                        nc.sync.dma_start(
                            out[o0 : o0 + P, y0 : y0 + RCH, :],
                            osb[:].rearrange("p (a c) -> p a c", c=W),
                        )
    nc.compile()
    return nc


_NC_CACHE = None


def _get_nc():
    global _NC_CACHE
    if _NC_CACHE is None:
        _NC_CACHE = build_nc()
    return _NC_CACHE


def make_in_maps(img, s, weight):
    """Host-side input prep: shard over batch, static weight transforms."""
    img = np.asarray(img, dtype=np.float32)
    s = np.ascontiguousarray(np.asarray(s, dtype=np.float32))
    weight = np.asarray(weight, dtype=np.float32)
    # zero-pad image host-side: [B, NI, P, HP, WP]
    imgp = np.zeros((img.shape[0], NI, P, HP, WP), dtype=np.float32)
    imgp[:, :, :, 1 : H + 1, 1 : W + 1] = img.reshape(-1, NI, P, H, W)
    # [O, I, ky, kx] -> [ky*kx, I, O] so lhsT tiles DMA contiguously
    wt = np.ascontiguousarray(
        weight.transpose(2, 3, 1, 0).reshape(KYX, CIN, COUT)
    )
    # A_T[i, o] = sum_k weight[o, i, :, :]^2  (static, sample-independent)
    at = np.ascontiguousarray(
        (weight.astype(np.float64) ** 2).sum(axis=(2, 3)).T.astype(np.float32)
    )
    return [
        {"img": imgp[b], "s": s[b], "wt": wt, "at": at} for b in range(N_CORES)
    ]


def kernel(img, s, weight):
    nc = _get_nc()
    in_maps = make_in_maps(img, s, weight)
    res = run_bass_kernel_spmd(nc, in_maps, list(range(N_CORES)))
    return np.stack([res.results[b]["out"] for b in range(N_CORES)], axis=0)



# revision 2
# speedup vs baseline: 41.9064x; 41.9064x over previous
"""Trainium2 Bass kernel v3 for Conv2Demod — Winograd F(2x2,3x3), bf16 PE.

Per-sample computation restructured as Winograd:
    out = A^T [ (G w G^T * (1+s_i) * d_o) elemwise (B^T d B) ] A
  - Host precomputes wg[u,v,i,o] = (G W G^T); the per-channel modulation
    (1+s_i) commutes with G and is applied on-device as a per-partition
    scale on GpSimd.  Demod d_o is folded into the PSUM eviction scale.
  - MAC count drops 2.25x vs direct conv: 16 (u,v) matmuls over 32x32
    output tiles instead of 9 taps over 64x64 pixels.
  - Input transform (B^T d B) and output transform (A^T M A) run on the
    DVE as tensor_tensor add/sub chains, overlapping the PE.  The padded
    image is stored x-DE-INTERLEAVED (even columns then odd columns) so
    every transform access has unit innermost stride, which qualifies the
    bf16 ops for the DVE 2x performance mode.

Pipeline per core (one sample): two y-halves of 16 tile-rows each;
per half: DVE transforms the image into V[it][uv, 512 tiles] (bf16),
then for each o-tile the PE runs 2 phases x 8 uv x 4 i-tile
accumulating matmuls (8 PSUM banks), ACT evicts with demod scale to
bf16 M, DVE applies the output transform into f32 osb, DMA to DRAM.
Sharding: one sample per NeuronCore, 8 cores data-parallel.
"""

import contextlib

import numpy as np
import ml_dtypes

import concourse.bacc as bacc
import concourse.mybir as mybir
import concourse.tile as tile
from concourse.bass_utils import run_bass_kernel_spmd

P = 128
CIN = 512
COUT = 512
H = W = 64
NI = CIN // P
NO = COUT // P
T = H // 2          # 32 tile rows/cols
TH = T // 2         # 16 tile rows per half
NPIX = TH * T       # 512 tiles per half
HP = H + 2
WP = W + 2          # 66 = 33 even + 33 odd x-positions
XE = WP // 2        # 33
EPS = 1e-8
N_CORES = 8

F32 = mybir.dt.float32
BF16 = mybir.dt.bfloat16
AF = mybir.ActivationFunctionType
ALU = mybir.AluOpType
_nullcm = contextlib.nullcontext


def build_nc(loop_n=None):
    nc = bacc.Bacc("TRN2", target_bir_lowering=False, debug=False)

    img = nc.dram_tensor("img", [NI, P, HP, WP], BF16, kind="ExternalInput").ap()
    s_in = nc.dram_tensor("s", [CIN], F32, kind="ExternalInput").ap()
    wg = nc.dram_tensor("wg", [NI, P, 16 * COUT], BF16, kind="ExternalInput").ap()
    at = nc.dram_tensor("at", [CIN, COUT], BF16, kind="ExternalInput").ap()
    out = nc.dram_tensor("out", [COUT, H, W], F32, kind="ExternalOutput").ap()

    with tile.TileContext(nc) as tc:
        with (
            tc.tile_pool(name="const", bufs=1) as cpool,
            tc.tile_pool(name="persist", bufs=1) as ppool,
            tc.tile_pool(name="imgh", bufs=5) as imghp,
            tc.tile_pool(name="v1", bufs=2) as v1p,
            tc.tile_pool(name="msb", bufs=1) as msbp,
            tc.tile_pool(name="psb", bufs=1) as psbp,
            tc.tile_pool(name="osb", bufs=2) as osbp,
            tc.tile_pool(name="psum", bufs=8, space="PSUM") as psum_pool,
        ):
            with (tc.For_i(0, loop_n, 1) if loop_n else _nullcm()):
                # ---- s-derived scalars -------------------------------------
                sraw = cpool.tile([P, NI, 2], F32, tag="sraw")
                for c in range(2):
                    nc.sync.dma_start(
                        sraw[:, :, c], s_in.rearrange("(t p) -> p t", p=P)
                    )
                smod = cpool.tile([P, NI, 2], F32, tag="smod")  # 1 + s
                nc.scalar.activation(smod[:], sraw[:], AF.Copy, bias=1.0)
                tsq = cpool.tile([P, NI, 2], BF16, tag="tsq")   # (1 + s)^2
                nc.scalar.square(tsq[:], smod[:])

                # ---- demod d[o] = 1/sqrt(A_T.T @ tsq + eps) ----------------
                at_sb = ppool.tile([P, NI, COUT], BF16, tag="at_sb")
                nc.sync.dma_start(
                    at_sb[:], at.rearrange("(t p) o -> p t o", p=P)
                )
                dsb = cpool.tile([P, NO], F32, tag="dsb")
                dtmp = cpool.tile([P, NO], F32, tag="dtmp")
                epst = cpool.tile([P, 1], F32, tag="epst")
                nc.vector.memset(epst[:], EPS)
                for ot in range(NO):
                    o0 = ot * P
                    psd = psum_pool.tile([P, NPIX], F32, tag="ps")
                    for it in range(NI):
                        nc.tensor.matmul(
                            psd[:, 0:2],
                            at_sb[:, it, o0 : o0 + P],
                            tsq[:, it, :],
                            start=(it == 0),
                            stop=(it == NI - 1),
                        )
                    nc.scalar.activation(
                        dtmp[:, ot : ot + 1], psd[:, 0:1], AF.Sqrt, bias=epst[:]
                    )
                nc.vector.reciprocal(dsb[:], dtmp[:])

                # ---- modulated transformed weights (GpSimd, in-place) ------
                wmod = []
                for it in range(NI):
                    wm = ppool.tile([P, 16, COUT], BF16, tag=f"wmod{it}")
                    wmod.append(wm)
                    nc.sync.dma_start(
                        wm[:].rearrange("p a b -> p (a b)"), wg[it]
                    )
                    nc.scalar.activation(
                        wm[:].rearrange("p a b -> p (a b)"),
                        wm[:].rearrange("p a b -> p (a b)"),
                        AF.Copy, scale=smod[:, it, 0:1],
                    )

                # ---- per half: input transform, matmuls, output transform --
                vt = []
                for it in range(NI):
                    v = ppool.tile([P, 16, TH, T], BF16, tag=f"vt{it}")
                    vt.append(v)

                def tt(o, a, b, op):
                    nc.vector.tensor_tensor(o, a, b, op)

                for h in range(2):
                    y0 = 2 * TH * h  # padded-image row base (=32h)
                    # -- input transform (u-major so PE phase A starts after
                    #    u=0,1 are done instead of after all four i-tiles) --
                    imghs = []
                    for it in range(NI):
                        imgh = imghp.tile([P, 17, 2, WP], BF16, tag="imgh")
                        imghs.append(imgh)
                        nc.sync.dma_start(
                            imgh[:].rearrange("p a b x -> p (a b) x"),
                            img[it][:, y0 : y0 + 34, :],
                        )
                    for u in range(4):
                        for it in range(NI):
                            imgh = imghs[it]
                            d0 = imgh[:, 0:TH, 0, :]
                            d1 = imgh[:, 0:TH, 1, :]
                            d2 = imgh[:, 1 : TH + 1, 0, :]
                            d3 = imgh[:, 1 : TH + 1, 1, :]
                            # BT: u0=d0-d2, u1=d1+d2, u2=d2-d1, u3=d1-d3
                            upat = [
                                (d0, d2, ALU.subtract),
                                (d1, d2, ALU.add),
                                (d2, d1, ALU.subtract),
                                (d1, d3, ALU.subtract),
                            ][u]
                            # v1 keeps the de-interleaved x layout
                            v1 = v1p.tile([P, TH, WP], BF16, tag="v1")
                            tt(v1[:], upat[0], upat[1], upat[2])
                            x0 = v1[:, :, 0:T]            # even x: 0..62
                            x2 = v1[:, :, 1 : T + 1]      # even x: 2..64
                            x1 = v1[:, :, XE : XE + T]    # odd x: 1..63
                            x3 = v1[:, :, XE + 1 : XE + T + 1]  # odd: 3..65
                            vpat = [
                                (x0, x2, ALU.subtract),
                                (x1, x2, ALU.add),
                                (x2, x1, ALU.subtract),
                                (x1, x3, ALU.subtract),
                            ]
                            for v, (xa, xb, xop) in enumerate(vpat):
                                tt(vt[it][:, u * 4 + v], xa, xb, xop)

                    # -- matmuls + output transform per o-tile --------------
                    for ot in range(NO):
                        o0 = ot * P
                        msb = msbp.tile([P, 16, NPIX], BF16, tag="msb")
                        for ph in range(2):
                            pss = []
                            for k in range(8):
                                ps = psum_pool.tile([P, NPIX], F32, tag="ps")
                                pss.append(ps)
                            for it in range(NI):
                                for k in range(8):
                                    uv = ph * 8 + k
                                    nc.tensor.matmul(
                                        pss[k][:],
                                        wmod[it][:, uv, o0 : o0 + P],
                                        vt[it][:, uv].rearrange(
                                            "p t c -> p (t c)"),
                                        start=(it == 0),
                                        stop=(it == NI - 1),
                                    )
                            for k in range(8):
                                nc.scalar.activation(
                                    msb[:, ph * 8 + k, :], pss[k][:],
                                    AF.Copy, scale=dsb[:, ot : ot + 1],
                                )
                        # output transform: AT = [[1,1,1,0],[0,1,-1,-1]]
                        psb = psbp.tile([P, 8, NPIX], BF16, tag="psb")
                        M = [msb[:, uv, :] for uv in range(16)]
                        for v in range(4):
                            p0 = psb[:, v, :]
                            p1 = psb[:, 4 + v, :]
                            tt(p0, M[0 * 4 + v], M[1 * 4 + v], ALU.add)
                            tt(p0, p0, M[2 * 4 + v], ALU.add)
                            tt(p1, M[1 * 4 + v], M[2 * 4 + v], ALU.subtract)
                            tt(p1, p1, M[3 * 4 + v], ALU.subtract)
                        osb = osbp.tile([P, TH, 2, T, 2], F32, tag="osb")
                        for a in range(2):
                            pa = [
                                psb[:, a * 4 + v, :].rearrange(
                                    "p (t c) -> p t c", c=T)
                                for v in range(4)
                            ]
                            yr = [osb[:, :, a, :, b] for b in range(2)]
                            tt(yr[0], pa[0], pa[1], ALU.add)
                            tt(yr[0], yr[0], pa[2], ALU.add)
                            tt(yr[1], pa[1], pa[2], ALU.subtract)
                            tt(yr[1], yr[1], pa[3], ALU.subtract)
                        nc.sync.dma_start(
                            out[o0 : o0 + P, 2 * TH * h : 2 * TH * (h + 1), :],
                            osb[:].rearrange("p t a c b -> p (t a) (c b)"),
                        )
    nc.compile()
    return nc


_NC_CACHE = None


def _get_nc():
    global _NC_CACHE
    if _NC_CACHE is None:
        _NC_CACHE = build_nc()
    return _NC_CACHE


_G = np.array(
    [[1, 0, 0], [0.5, 0.5, 0.5], [0.5, -0.5, 0.5], [0, 0, 1]], np.float64
)


def make_in_maps(img, s, weight):
    img = np.asarray(img, dtype=np.float32)
    s = np.ascontiguousarray(np.asarray(s, dtype=np.float32))
    weight = np.asarray(weight, dtype=np.float32)
    # zero-pad, then DE-INTERLEAVE x: [even cols | odd cols]
    imgp = np.zeros((img.shape[0], NI, P, HP, WP), dtype=np.float32)
    imgp[:, :, :, 1 : H + 1, 1 : W + 1] = img.reshape(-1, NI, P, H, W)
    imgd = np.concatenate(
        [imgp[..., 0::2], imgp[..., 1::2]], axis=-1
    ).astype(ml_dtypes.bfloat16)
    # wg[u,v,i,o] = (G W G^T)[o,i,u,v] -> [NI, P, (u,v,o)]
    wgf = np.einsum("ua,oiab,vb->uvio", _G, weight.astype(np.float64), _G)
    wgt = np.ascontiguousarray(
        wgf.transpose(2, 0, 1, 3)          # [i, u, v, o]
        .reshape(NI, P, 16 * COUT)
        .astype(ml_dtypes.bfloat16)
    )
    at = np.ascontiguousarray(
        (weight.astype(np.float64) ** 2)
        .sum(axis=(2, 3))
        .T.astype(ml_dtypes.bfloat16)
    )
    return [
        {"img": imgd[b], "s": s[b], "wg": wgt, "at": at} for b in range(N_CORES)
    ]


def kernel(img, s, weight):
    nc = _get_nc()
    in_maps = make_in_maps(img, s, weight)
    res = run_bass_kernel_spmd(nc, in_maps, list(range(N_CORES)))
    return np.stack([res.results[b]["out"] for b in range(N_CORES)], axis=0)
